# Initial kernel scaffold
#
"""Trainium2 Bass kernel for nn_CASST (dense transformer, CTMF blocks).

Self-contained: builds the Bass program from the concrete numpy inputs,
shards batch B=128 across 8 NeuronCores (16 samples each), runs SPMD,
gathers the full [128, 16] output.

Per-core layout:
  X [128, 36, 512] f32 token-major residual: tiles 0..31 = spatial stream
  (16 samples x 256 padded rows, 226 valid: cls at row 0, patches 1..225),
  tiles 32..35 = spectral stream (16 samples x 32 padded rows, 31 valid).
  Matmul operands bf16; PSUM accumulation f32.
  Attention: scores computed transposed (keys on partitions) so softmax
  needs no transpose of the attention matrix; the row-sum comes free from
  an extra all-ones column appended to V, and the normalization is fused
  into the PSUM->SBUF copy of the per-head output.
"""
import sys

sys.path.insert(0, "/opt/trn_rl_repo")

import numpy as np
import ml_dtypes

import concourse.bass as bass
import concourse.tile as tile
from concourse import bacc
from concourse import mybir
from concourse.masks import make_identity
from concourse.bass_utils import run_bass_kernel_spmd

F32 = mybir.dt.float32
BF16 = mybir.dt.bfloat16
AF = mybir.ActivationFunctionType
ALU = mybir.AluOpType

B, BANDS, HW, DIM, NH, NCLS = 128, 30, 15, 512, 8, 16
NCORES = 8
SB = B // NCORES          # 16 samples per core
NPATCH = HW * HW          # 225
SPA_PAD, SPE_PAD = 256, 32
NT_SPA = SB * SPA_PAD // 128   # 32
NT_SPE = SB * SPE_PAD // 128   # 4
NT = NT_SPA + NT_SPE           # 36
EPS = 1e-5
HD = DIM // NH            # 64


def _bf(x):
    return np.asarray(x, dtype=np.float32).astype(ml_dtypes.bfloat16)


def _pack_kT(w_T, mdim):
    """[512, M] (contraction rows) -> [128, 4, M] (partition, k-chunk, M)."""
    k = w_T.shape[0]
    return np.ascontiguousarray(w_T.reshape(k // 128, 128, mdim).transpose(1, 0, 2))


def _ident(a, b):
    return bool(np.all(np.asarray(a) == 1) and np.all(np.asarray(b) == 0))


def prepare_weights(inp):
    w = {}
    # spatial conv + BN fold
    s_h = inp["bn_h_g"] / np.sqrt(inp["bn_h_v"] + EPS)
    b_h = inp["conv_h_b"] * s_h + inp["bn_h_b"] - inp["bn_h_m"] * s_h
    w_h = np.asarray(inp["conv_h_w"]) * np.asarray(s_h)[:, None, None, None]
    # K-tile ky: row 32*kx + b holds tap (ky,kx) band b; tile0 row 96 = bias
    w_spa = np.zeros((3, 128, DIM), np.float32)
    for ky in range(3):
        for kx in range(3):
            w_spa[ky, 32 * kx:32 * kx + 30, :] = np.asarray(w_h)[:, :, ky, kx].T
    w_spa[0, 96, :] = np.asarray(b_h)
    w["w_spa"] = _bf(w_spa)

    # spectral conv + BN fold
    s_c = inp["cnn_bn_g"] / np.sqrt(inp["cnn_bn_v"] + EPS)
    b_c = inp["cnn_conv_b"] * s_c + inp["cnn_bn_b"] - inp["cnn_bn_m"] * s_c
    w_c = np.asarray(inp["cnn_conv_w"]) * np.asarray(s_c)[:, None, None, None]
    # row 32*kx + ky holds tap (ky,kx) so border memsets stay 32-aligned
    w_cnn = np.zeros((67, 128), np.float32)
    for ky in range(3):
        for kx in range(3):
            w_cnn[32 * kx + ky, :] = w_c[:, 0, ky, kx]
    w["w_cnn"] = _bf(w_cnn)
    w["b_cnn"] = np.asarray(b_c, np.float32).reshape(128, 1)
    w["w_fc"] = _bf(np.asarray(inp["cnn_fc_w"]).T / NPATCH)   # mean folded
    w["fc_b"] = np.asarray(inp["cnn_fc_b"], np.float32)

    for i in range(2):
        qkv = np.asarray(inp["blk_qkv_w"][i])
        wq, wk, wv = qkv[:DIM], qkv[DIM:2 * DIM], qkv[2 * DIM:]
        w[f"wqk{i}"] = _bf(_pack_kT(np.concatenate([wq.T, wk.T], 1), 2 * DIM))
        w[f"wv{i}"] = _bf(_pack_kT(wv.T, DIM))
        w[f"wproj{i}"] = _bf(_pack_kT(np.asarray(inp["blk_proj_w"][i]).T, DIM))
        w[f"wfc1{i}"] = _bf(_pack_kT(np.asarray(inp["blk_fc1_w"][i]).T, DIM))
        w[f"wfc2{i}"] = _bf(_pack_kT(np.asarray(inp["blk_fc2_w"][i]).T, DIM))
    w["whead"] = _bf(_pack_kT(np.asarray(inp["head_w"]).T, NCLS))

    for k in ("blk_qkv_b", "blk_proj_b", "blk_fc1_b", "blk_fc2_b", "head_b",
              "blk_n1_w", "blk_n1_b", "blk_n2_w", "blk_n2_b",
              "norm1_w", "norm1_b", "norm2_w", "norm2_b"):
        w[k] = np.asarray(inp[k], np.float32)

    # pos_spa[p, c] = positional embedding for the token that conv-psum row p
    # of chunk c produces: chunk0 row p -> token p+1, chunk1 row p -> token 128+p
    pos_shift = np.zeros((128, 2, DIM), np.float32)
    spa_pos = np.asarray(inp["spa_pos"])[0]           # [226, 512]
    pos_shift[0:127, 0] = spa_pos[1:128]
    pos_shift[0:98, 1] = spa_pos[128:226]
    w["pos_spa"] = pos_shift
    pos_spe = np.zeros((SPE_PAD, DIM), np.float32)
    pos_spe[1:1 + BANDS] = np.asarray(inp["spe_pos"])[0, 1:1 + BANDS]
    w["pos_spe"] = np.ascontiguousarray(np.tile(pos_spe, (4, 1)))
    cls2 = np.zeros((2, DIM), np.float32)
    cls2[0] = np.asarray(inp["spa_cls"])[0, 0] + np.asarray(inp["spa_pos"])[0, 0]
    cls2[1] = np.asarray(inp["spe_cls"])[0, 0] + np.asarray(inp["spe_pos"])[0, 0]
    w["cls2"] = cls2

    w["cfg"] = dict(
        use_qkv_b=bool(np.any(w["blk_qkv_b"] != 0)),
        use_proj_b=bool(np.any(w["blk_proj_b"] != 0)),
        use_fc1_b=bool(np.any(w["blk_fc1_b"] != 0)),
        use_fc2_b=bool(np.any(w["blk_fc2_b"] != 0)),
        use_fc_b=bool(np.any(w["fc_b"] != 0)),
        use_head_b=bool(np.any(w["head_b"] != 0)),
        use_n1=not all(_ident(w["blk_n1_w"][i], w["blk_n1_b"][i]) for i in range(2)),
        use_n2=not all(_ident(w["blk_n2_w"][i], w["blk_n2_b"][i]) for i in range(2)),
        use_nf=not (_ident(w["norm1_w"], w["norm1_b"])
                    and _ident(w["norm2_w"], w["norm2_b"])),
    )
    return w


def _im2cols(xc):
    """Host im2col for one core's x shard [SB, 30, 15, 15] (f32)."""
    xp = np.pad(xc, ((0, 0), (0, 0), (1, 1), (1, 1)))
    im_spa = np.zeros((3, 128, SB * NPATCH), np.float32)
    im_spe = np.zeros((67, SB * BANDS * NPATCH), np.float32)
    for ky in range(3):
        for kx in range(3):
            win = xp[:, :, ky:ky + HW, kx:kx + HW]          # [SB,30,15,15]
            im_spa[ky, 32 * kx:32 * kx + 30, :] = (
                win.transpose(1, 0, 2, 3).reshape(BANDS, -1))
            im_spe[32 * kx + ky, :] = win.reshape(-1)
    im_spa[0, 96, :] = 1.0
    return _bf(im_spa), _bf(im_spe)


def make_in_maps(inputs, w):
    x = np.asarray(inputs["x"], np.float32)[:, 0]   # [128, 30, 15, 15]
    cfg = w["cfg"]
    base = {k: w[k] for k in
            ("w_spa", "w_cnn", "b_cnn", "w_fc", "whead", "pos_spa",
             "pos_spe", "cls2")}
    for i in range(2):
        for nm in ("wqk", "wv", "wproj", "wfc1", "wfc2"):
            base[nm + str(i)] = w[nm + str(i)]
    if cfg["use_qkv_b"]:
        base["qkv_b"] = np.ascontiguousarray(
            w["blk_qkv_b"].reshape(2, 12, 128).transpose(0, 2, 1))
    if cfg["use_proj_b"]:
        base["proj_b"] = w["blk_proj_b"]
    if cfg["use_fc1_b"]:
        base["fc1_b"] = np.ascontiguousarray(
            w["blk_fc1_b"].reshape(2, 4, 128).transpose(0, 2, 1))
    if cfg["use_fc2_b"]:
        base["fc2_b"] = w["blk_fc2_b"]
    if cfg["use_fc_b"]:
        base["fc_b"] = w["fc_b"].reshape(1, DIM)
    if cfg["use_head_b"]:
        base["head_b"] = w["head_b"].reshape(1, NCLS)
    if cfg["use_n1"]:
        base["n1_wb"] = np.ascontiguousarray(
            np.stack([w["blk_n1_w"], w["blk_n1_b"]], axis=1))
    if cfg["use_n2"]:
        base["n2_wb"] = np.ascontiguousarray(
            np.stack([w["blk_n2_w"], w["blk_n2_b"]], axis=1))
    if cfg["use_nf"]:
        base["nf_w"] = np.stack([w["norm1_w"], w["norm2_w"]])
        base["nf_b"] = np.stack([w["norm1_b"], w["norm2_b"]])
    maps = []
    for c in range(NCORES):
        m = dict(base)
        m["im_spa"], m["im_spe"] = _im2cols(x[c * SB:(c + 1) * SB])
        maps.append(m)
    return maps


# =====================================================================
def build_program(w, debug_stage=None):
    cfg = w["cfg"]
    nc = bacc.Bacc(None)
    P = {}

    def dparam(name, shape, dt):
        P[name] = nc.declare_dram_parameter(name, list(shape), dt, isOutput=False)

    dparam("im_spa", (3, 128, SB * NPATCH), BF16)
    dparam("im_spe", (67, SB * BANDS * NPATCH), BF16)
    dparam("w_spa", (3, 128, DIM), BF16)
    dparam("w_cnn", (67, 128), BF16)
    dparam("b_cnn", (128, 1), F32)
    dparam("w_fc", (128, DIM), BF16)
    for i in range(2):
        dparam(f"wqk{i}", (128, 4, 2 * DIM), BF16)
        for nm in ("wv", "wproj", "wfc1", "wfc2"):
            dparam(f"{nm}{i}", (128, 4, DIM), BF16)
    dparam("whead", (128, 8, NCLS), BF16)
    dparam("pos_spa", (128, 2, DIM), F32)
    dparam("pos_spe", (128, DIM), F32)
    dparam("cls2", (2, DIM), F32)
    if cfg["use_qkv_b"]:
        dparam("qkv_b", (2, 128, 12), F32)
    if cfg["use_proj_b"]:
        dparam("proj_b", (2, DIM), F32)
    if cfg["use_fc1_b"]:
        dparam("fc1_b", (2, 128, 4), F32)
    if cfg["use_fc2_b"]:
        dparam("fc2_b", (2, DIM), F32)
    if cfg["use_fc_b"]:
        dparam("fc_b", (1, DIM), F32)
    if cfg["use_head_b"]:
        dparam("head_b", (1, NCLS), F32)
    if cfg["use_n1"]:
        dparam("n1_wb", (2, 2, DIM), F32)
    if cfg["use_n2"]:
        dparam("n2_wb", (2, 2, DIM), F32)
    if cfg["use_nf"]:
        dparam("nf_w", (2, DIM), F32)
        dparam("nf_b", (2, DIM), F32)
    out_p = nc.declare_dram_parameter("out", [SB, NCLS], F32, isOutput=True)
    dbg_p = None
    if debug_stage is not None:
        dbg_p = nc.declare_dram_parameter("dbgX", [128, NT, DIM], F32,
                                          isOutput=True)

    with tile.TileContext(nc) as tc:
        Kernel(tc, P, out_p, cfg, debug_stage, dbg_p).build()
    nc.finalize()   # Bacc: runs wait-splitting legalization + reg alloc
    return nc


class Kernel:
    def __init__(self, tc, P, out_p, cfg, debug_stage=None, dbg_p=None):
        self.tc, self.nc, self.P, self.out_p, self.cfg = tc, tc.nc, P, out_p, cfg
        self.debug_stage, self.dbg_p = debug_stage, dbg_p

    def dbg_dump(self, stage):
        if self.debug_stage == stage:
            self.nc.sync.dma_start(out=self.dbg_p[:], in_=self.X)

    def dbg_dump_tile(self, stage, ap):
        """Dump an arbitrary [128, N] SBUF tile into dbgX[:, 0, :N]."""
        if self.debug_stage == stage:
            n = ap.shape[-1]
            self.nc.sync.dma_start(out=self.dbg_p[:, 0, 0:n], in_=ap)

    def build(self):
        tc, nc, P = self.tc, self.nc, self.P
        with tc.tile_pool(name="const", bufs=1) as cp:
            self.cp = cp
            X = cp.tile([128, NT, DIM], F32, name="X")
            self.X = X
            nc.vector.memset(X, 0.0)
            ident_bf = cp.tile([128, 128], BF16, name="ident_bf")
            make_identity(nc, ident_bf)
            self.ident = ident_bf
            eps_sb = cp.tile([128, 1], F32, name="eps_sb")
            nc.vector.memset(eps_sb, EPS)
            self.eps = eps_sb

            def load(name, shape, dt, src=None):
                t = cp.tile(list(shape), dt, name="sb_" + name)
                nc.sync.dma_start(out=t, in_=src if src is not None else P[name][:])
                return t

            self.w_spa = load("w_spa", (128, 3, DIM), BF16,
                              P["w_spa"][:].rearrange("a p m -> p a m"))
            self.w_cnn = load("w_cnn", (67, 128), BF16)
            self.b_cnn = load("b_cnn", (128, 1), F32)
            self.w_fc = load("w_fc", (128, DIM), BF16)
            self.whead = load("whead", (128, 8, NCLS), BF16)
            self.pos_spa = load("pos_spa", (128, 2, DIM), F32)
            self.pos_spe = load("pos_spe", (128, DIM), F32)
            self.cls2 = load("cls2", (1, 2, DIM), F32,
                             P["cls2"][:].rearrange("a d -> 1 a d") if False
                             else P["cls2"][None, :, :])
            self.bias = {}
            for k, shp in (("qkv_b", (2, 128, 12)), ("proj_b", (2, DIM)),
                           ("fc1_b", (2, 128, 4)), ("fc2_b", (2, DIM)),
                           ("fc_b", (1, DIM)), ("head_b", (1, NCLS)),
                           ("n1_wb", (2, 2, DIM)), ("n2_wb", (2, 2, DIM)),
                           ("nf_w", (2, DIM)), ("nf_b", (2, DIM))):
                if k in P:
                    self.bias[k] = load(k, shp, F32)

            self.pool_sb = cp.tile([128, SB * SPE_PAD], F32, name="pool_sb")
            nc.vector.memset(self.pool_sb, 0.0)

            with tc.tile_pool(name="convp", bufs=1) as cvp, \
                 tc.tile_pool(name="convtmp", bufs=3) as cvt, \
                 tc.tile_pool(name="convps", bufs=1, space="PSUM") as cps:
                self.conv_stage(cvp, cvt, cps)

            self.dbg_dump(0)
            with tc.tile_pool(name="wblk", bufs=1) as wp, \
                 tc.tile_pool(name="blk", bufs=1) as bp, \
                 tc.tile_pool(name="grp", bufs=2) as gp, \
                 tc.tile_pool(name="attn", bufs=4) as ap, \
                 tc.tile_pool(name="small", bufs=4) as sp, \
                 tc.tile_pool(name="ps", bufs=1, space="PSUM") as ps:
                self.wp, self.bp, self.gp, self.ap, self.sp, self.ps = \
                    wp, bp, gp, ap, sp, ps
                for i in range(2):
                    if self.debug_stage is not None and self.debug_stage <= i:
                        break
                    self.block(i)
                    self.dbg_dump(i + 1)
                self.head()

    # psum helpers: one pool, explicit per-tag bufs (total <= 8 banks)
    def ps_mm(self):
        return self.ps.tile([128, DIM], F32, tag="mm", bufs=2, name="ps_mm")

    def ps_tr(self):
        return self.ps.tile([128, 128], BF16, tag="tr", bufs=2, name="ps_tr")

    def ps_sT(self):
        return self.ps.tile([128, DIM], F32, tag="sT", bufs=2, name="ps_sT")

    def ps_o(self):
        return self.ps.tile([128, 4 * 65], F32, tag="ops", bufs=2, name="ps_o")

    # ------------------------------------------------------------ conv
    def conv_stage(self, cvp, cvt, cps):
        nc, X = self.nc, self.X
        # host-built im2cols, plain DMA loads
        im = [cvp.tile([97, SB, NPATCH], BF16, name=f"im_spa{k}")
              for k in range(3)]
        for k in range(3):
            nc.sync.dma_start(
                out=im[k],
                in_=self.P["im_spa"][k, 0:97].rearrange(
                    "k (s p) -> k s p", s=SB))
        imf = im

        for s in range(SB):
            for ci, (p0, p1) in enumerate(((0, 127), (127, 225))):
                m = p1 - p0
                psm = cps.tile([128, DIM], F32, tag="spaps", bufs=3,
                               name="psm_spa")
                for k in range(3):
                    kv = 97 if k == 0 else 94
                    nc.tensor.matmul(psm[:m], imf[k][:kv, s, p0:p1],
                                     self.w_spa[:kv, k, :],
                                     start=(k == 0), stop=(k == 2))
                tmp = cvt.tile([128, DIM], F32, tag="spatmp", name="tmp_spa")
                nc.scalar.activation(out=tmp[:m], in_=psm[:m], func=AF.Relu)
                nc.vector.tensor_tensor(out=tmp[:m], in0=tmp[:m],
                                        in1=self.pos_spa[:m, ci, :],
                                        op=ALU.add)
                if ci == 0:
                    # token rows 1..127 of tile 2s: partition base 1 is not
                    # engine-addressable -> bounce through DMA
                    nc.sync.dma_start(out=X[1:128, 2 * s, :], in_=tmp[:m])
                else:
                    nc.vector.tensor_copy(out=X[0:98, 2 * s + 1, :],
                                          in_=tmp[:m])

        # spectral: chunks of 15 instances (half sample); tap row = 32*kx+ky
        im2 = [cvp.tile([67, 15, NPATCH], BF16, name=f"im_spe{k}")
               for k in range(3)]
        im_spe_p = self.P["im_spe"][:].rearrange("k (i p) -> k i p", p=NPATCH)
        ninst = 0
        for cc in range(SB * 2):
            s, h2 = cc // 2, cc % 2
            t = im2[cc % 3]
            i_base = 30 * s + 15 * h2
            nc.sync.dma_start(out=t, in_=im_spe_p[:, i_base:i_base + 15, :])
            tf = t
            for g in range(8):
                i0, i1 = 2 * g, min(2 * g + 2, 15)
                n = (i1 - i0) * NPATCH
                psm = cps.tile([128, 2 * NPATCH], F32, tag="speps", bufs=3,
                               name="psm_spe")
                nc.tensor.matmul(psm[:, :n], self.w_cnn[:67, :],
                                 tf[:67, i0:i1, :], start=True, stop=True)
                ni = i1 - i0
                col0 = SPE_PAD * s + 1 + 15 * h2 + i0
                if (cc * 8 + g) % 3 == 2:
                    # DVE path: relu whole tile, then reduce per instance
                    # (DVE tensor_scalar accum_out is broken on HW)
                    relu_d = cvt.tile([128, 2 * NPATCH], BF16, tag="relu_d",
                                      name="relu_d")
                    nc.vector.tensor_scalar(
                        out=relu_d[:, :n], in0=psm[:, :n], scalar1=self.b_cnn,
                        scalar2=0.0, op0=ALU.add, op1=ALU.max)
                    nc.vector.reduce_sum(
                        out=self.pool_sb[:, col0:col0 + ni],
                        in_=relu_d[:, :n].rearrange("p (i q) -> p i q",
                                                    q=NPATCH),
                        axis=mybir.AxisListType.X)
                    ninst += ni
                else:
                    for li in range(ni):
                        col = col0 + li
                        src = psm[:, li * NPATCH:(li + 1) * NPATCH]
                        trash_a = cvt.tile([128, NPATCH], F32, tag="trash_a",
                                           name="trash_a")
                        nc.scalar.activation(
                            out=trash_a, in_=src, func=AF.Relu,
                            bias=self.b_cnn, scale=1.0,
                            accum_out=self.pool_sb[:, col:col + 1])
                        ninst += 1

        self.dbg_dump_tile(10, self.pool_sb)
        pool_bf = cvp.tile([128, SB * SPE_PAD], BF16, name="pool_bf")
        nc.vector.tensor_copy(out=pool_bf, in_=self.pool_sb)
        for g in range(4):
            psm = cps.tile([128, DIM], F32, tag="fcps", bufs=2, name="psm_fc")
            nc.tensor.matmul(psm, pool_bf[:, 128 * g:128 * (g + 1)], self.w_fc,
                             start=True, stop=True)
            tmpf = cvt.tile([128, DIM], BF16, tag="fctmp", name="tmp_fc")
            nc.scalar.activation(out=tmpf, in_=psm, func=AF.Relu)
            if self.cfg["use_fc_b"]:
                nc.vector.tensor_tensor(
                    out=tmpf, in0=tmpf,
                    in1=self.bias["fc_b"][0:1, :].to_broadcast((1, DIM)),
                    op=ALU.add)
            nc.vector.tensor_tensor(out=X[:, NT_SPA + g, :], in0=tmpf,
                                    in1=self.pos_spe, op=ALU.add)

        # cls tokens
        nc.vector.tensor_copy(
            out=X[0:1, 0:NT_SPA:2, :],
            in_=self.cls2[0:1, 0:1, :].to_broadcast((1, SB, DIM)))
        for k in range(4):
            nc.vector.tensor_copy(
                out=X[32 * k:32 * k + 1, NT_SPA:NT, :],
                in_=self.cls2[0:1, 1:2, :].to_broadcast((1, 4, DIM)))

    # ------------------------------------------------------------ layernorm
    def ln_stats(self, stats, c0, c1):
        """Stats for chunks [c0, c1); batched invstd = exp(-0.5*ln(var+eps))
        (strided over the range — per-chunk ACT ops thrashed the table set,
        a single full-tile op serialized the block start)."""
        nc, X = self.nc, self.X
        for c in range(c0, c1):
            st = self.sp.tile([128, 6], F32, tag="lnst", name="st")
            nc.vector.bn_stats(out=st, in_=X[:, c, :])
            nc.vector.bn_aggr(out=stats[:, c, :], in_=st)
        v = stats[:, c0:c1, 1:2]
        nc.scalar.activation(out=v, in_=v, func=AF.Ln, bias=self.eps,
                             scale=1.0)
        nc.scalar.activation(out=v, in_=v, func=AF.Exp, scale=-0.5)

    def ln_apply_T(self, stats, c, dst, dst_col, affine=None):
        """LN chunk c -> transpose -> dst[:, e, dst_col:+128] (bf16)."""
        nc, X = self.nc, self.X
        lno = self.sp.tile([128, DIM], BF16, tag="lno", name="lno")
        nc.vector.tensor_scalar(out=lno, in0=X[:, c, :],
                                scalar1=stats[:, c, 0:1],
                                scalar2=stats[:, c, 1:2],
                                op0=ALU.subtract, op1=ALU.mult)
        if affine is not None:
            nc.vector.tensor_tensor(out=lno, in0=lno,
                                    in1=affine[0:1, :].to_broadcast((1, DIM)),
                                    op=ALU.mult)
            nc.vector.tensor_tensor(out=lno, in0=lno,
                                    in1=affine[1:2, :].to_broadcast((1, DIM)),
                                    op=ALU.add)
        for e in range(4):
            pt = self.ps_tr()
            nc.tensor.transpose(pt, lno[:, 128 * e:128 * (e + 1)], self.ident)
            nc.any.tensor_copy(out=dst[:, e, dst_col:dst_col + 128], in_=pt)

    # ------------------------------------------------------------ block
    def qkv_group(self, blk, xlnT_g, q_g, k_g, v_g):
        nc = self.nc
        wqk, wv = self.wblk[f"wqk{blk}"], self.wblk[f"wv{blk}"]
        qkv_b = self.bias.get("qkv_b")
        for m in range(8):
            psm = self.ps_mm()
            for kc in range(4):
                nc.tensor.matmul(psm, wqk[:, kc, 128 * m:128 * (m + 1)],
                                 xlnT_g[:, kc, :], start=(kc == 0),
                                 stop=(kc == 3))
            dst = q_g[:, m, :] if m < 4 else k_g[:, m - 4, :]
            if m < 4:
                if qkv_b is not None:
                    nc.vector.tensor_scalar(out=dst, in0=psm,
                                            scalar1=qkv_b[blk][:, m:m + 1],
                                            scalar2=float(HD) ** -0.5,
                                            op0=ALU.add, op1=ALU.mult)
                else:
                    nc.vector.tensor_scalar(out=dst, in0=psm,
                                            scalar1=float(HD) ** -0.5,
                                            scalar2=None, op0=ALU.mult)
            else:
                if qkv_b is not None:
                    nc.vector.tensor_scalar(out=dst, in0=psm,
                                            scalar1=qkv_b[blk][:, m:m + 1],
                                            scalar2=None, op0=ALU.add)
                else:
                    nc.vector.tensor_copy(out=dst, in_=psm)
        for t in range(4):
            psm = self.ps_mm()
            for kc in range(4):
                nc.tensor.matmul(psm, xlnT_g[:, kc, 128 * t:128 * (t + 1)],
                                 wv[:, kc, :], start=(kc == 0), stop=(kc == 3))
            nc.vector.tensor_copy(out=v_g[:, t, :, 0:64], in_=psm)
        nc.vector.memset(v_g[:, :, :, 64:65], 1.0)

    def attn_sample(self, q_g, k_g, v_g, oT_g, n0, nw, mchunks, nchunks):
        """mchunks: [(tile, base, rows)]; nchunks: [(col0, rows)]."""
        nc = self.nc
        o_sb = self.ap.tile([128, 2, DIM], BF16, tag="osb", name="o_sb")
        for og in range(2):
            o_ps = [self.ps_o() for _ in nchunks]
            for hh in range(4):
                h = 4 * og + hh
                hp, hc = 64 * (h % 2), h // 2
                aTx = self.ap.tile([128, 2, 256], BF16, tag="aTx", name="aTx")
                if len(mchunks) == 2 and nw == 256:
                    # both m-chunks into one PSUM bank -> single exp op
                    # (chunk1 rows mk..127 hold stale data; excluded by the
                    # K-slice of the o-matmul, so exp of them is harmless)
                    pss = self.ps_sT()
                    for mi, (mt, mb, mk) in enumerate(mchunks):
                        nc.tensor.matmul(
                            pss[:mk, 256 * mi:256 * mi + 256],
                            k_g[hp:hp + 64, hc,
                                128 * mt + mb:128 * mt + mb + mk],
                            q_g[hp:hp + 64, hc, n0:n0 + nw],
                            start=True, stop=True)
                    nc.scalar.activation(
                        out=aTx.rearrange("p a b -> p (a b)"),
                        in_=pss, func=AF.Exp)
                else:
                    for mi, (mt, mb, mk) in enumerate(mchunks):
                        pss = self.ps_sT()
                        nc.tensor.matmul(
                            pss[:mk, :nw],
                            k_g[hp:hp + 64, hc,
                                128 * mt + mb:128 * mt + mb + mk],
                            q_g[hp:hp + 64, hc, n0:n0 + nw],
                            start=True, stop=True)
                        nc.scalar.activation(out=aTx[mb:mb + mk, mi, :nw],
                                             in_=pss[:mk, :nw], func=AF.Exp)
                for ni, (nc0, nr) in enumerate(nchunks):
                    for mi, (mt, mb, mk) in enumerate(mchunks):
                        nc.tensor.matmul(
                            o_ps[ni][:nr, 65 * hh:65 * hh + 65],
                            aTx[mb:mb + mk, mi, nc0:nc0 + nr],
                            v_g[mb:mb + mk, mt, h, :],
                            start=(mi == 0), stop=(mi == len(mchunks) - 1),
                            tile_position=(mb if mk <= 32 else 0, 0))
            for ni, (nc0, nr) in enumerate(nchunks):
                rinv = self.sp.tile([128, 8], F32, tag="rinv", name="rinv")
                nc.vector.reciprocal(out=rinv[:nr, 4 * og:4 * og + 4],
                                     in_=o_ps[ni][:nr, 64:260:65])
                # one op for all 4 heads: broadcast 1/rowsum over head dim
                src = o_ps[ni][:nr, :260].rearrange(
                    "p (h e) -> p h e", e=65)[:, :, 0:64]
                dst = o_sb[:nr, ni, 256 * og:256 * og + 256].rearrange(
                    "p (h e) -> p h e", e=64)
                nc.vector.tensor_tensor(
                    out=dst, in0=src,
                    in1=rinv[:nr, 4 * og:4 * og + 4, None].to_broadcast(
                        (nr, 4, 64)),
                    op=ALU.mult)
        # transpose o (token-major) -> oT_g feature-major columns
        for ni, (nc0, nr) in enumerate(nchunks):
            for e in range(4):
                pt = self.ps_tr()
                nc.tensor.transpose(pt[:, :nr], o_sb[:nr, ni, 128 * e:128 * (e + 1)],
                                    self.ident[:nr, :nr])
                nc.any.tensor_copy(
                    out=oT_g[:, e, n0 + nc0:n0 + nc0 + nr], in_=pt[:, :nr])

    def proj_group(self, blk, oT_g, base_tile):
        """Blanket residual: X[:, tile, :] += proj(oT). cls rows get the
        wrong (own-stream) delta here; fixed afterwards via cls_fix()."""
        nc, X = self.nc, self.X
        wproj = self.wblk[f"wproj{blk}"]
        for cc in range(4):
            psz = self.ps_mm()
            for e in range(4):
                nc.tensor.matmul(psz, oT_g[:, e, 128 * cc:128 * (cc + 1)],
                                 wproj[:, e, :], start=(e == 0), stop=(e == 3))
            if self.cfg["use_proj_b"]:
                nc.vector.tensor_tensor(
                    out=psz, in0=psz,
                    in1=self.bias["proj_b"][blk:blk + 1, :].to_broadcast((1, DIM)),
                    op=ALU.add)
            dt = base_tile + cc
            nc.vector.tensor_tensor(out=X[:, dt, :], in0=X[:, dt, :],
                                    in1=psz, op=ALU.add)

    def gather_cls(self, tx, ty):
        """DMA-gather the 32 cls rows of X into [16,512] tiles (spa, spe)."""
        nc, X = self.nc, self.X
        nc.sync.dma_start(out=tx, in_=X[0:1, 0:NT_SPA:2, :])
        for k in range(4):
            nc.sync.dma_start(out=ty[k:16:4, :],
                              in_=X[32 * k:32 * k + 1, NT_SPA:NT, :])

    def scatter_cls(self, tx, ty):
        nc, X = self.nc, self.X
        nc.sync.dma_start(out=X[0:1, 0:NT_SPA:2, :], in_=tx)
        for k in range(4):
            nc.sync.dma_start(out=X[32 * k:32 * k + 1, NT_SPA:NT, :],
                              in_=ty[k:16:4, :])

    def block(self, blk):
        nc, X, tc = self.nc, self.X, self.tc
        cfg = self.cfg
        # per-block weights
        self.wblk = {}
        for nm in ("wqk", "wv", "wproj", "wfc1", "wfc2"):
            key = nm + str(blk)
            shape = [128, 4, 2 * DIM] if nm == "wqk" else [128, 4, DIM]
            t = self.wp.tile(shape, BF16, tag=nm, name="w_" + key)
            nc.sync.dma_start(out=t, in_=self.P[key][:])
            self.wblk[key] = t

        aff1 = self.bias["n1_wb"][blk] if cfg["use_n1"] else None
        stats1 = self.bp.tile([128, NT, 2], F32, tag="stats1", name="stats1")
        self.ln_stats(stats1, 0, NT)
        # snapshot cls rows before any residual update (for the cls fix)
        xc0_x = self.bp.tile([16, DIM], F32, tag="xc0x", name="xc0_x")
        xc0_y = self.bp.tile([16, DIM], F32, tag="xc0y", name="xc0_y")
        self.gather_cls(xc0_x, xc0_y)

        xlnT_spe = self.bp.tile([128, 4, DIM], BF16, tag="xlnTspe",
                                name="xlnT_spe")
        for lc in range(4):
            self.ln_apply_T(stats1, NT_SPA + lc, xlnT_spe, 128 * lc, aff1)

        spa_nchunks = [(0, 128), (128, 128)]
        for g in range(8):
            xlnT_g = self.gp.tile([128, 4, DIM], BF16, tag="xlnT", name="xlnT_g")
            for lc in range(4):
                self.ln_apply_T(stats1, 4 * g + lc, xlnT_g, 128 * lc, aff1)
            for j in range(2):
                s = 2 * g + j
                ctmp = self.sp.tile([128, 4, 1], BF16, tag="ctmp", name="ctmp")
                nc.sync.dma_start(out=ctmp,
                                  in_=xlnT_spe[:, :, 32 * s:32 * s + 1])
                nc.sync.dma_start(out=xlnT_spe[:, :, 32 * s:32 * s + 1],
                                  in_=xlnT_g[:, :, 256 * j:256 * j + 1])
                nc.sync.dma_start(out=xlnT_g[:, :, 256 * j:256 * j + 1],
                                  in_=ctmp)
            q_g = self.gp.tile([128, 4, DIM], BF16, tag="qg", name="q_g")
            k_g = self.gp.tile([128, 4, DIM], BF16, tag="kg", name="k_g")
            v_g = self.gp.tile([128, 4, NH, 65], BF16, tag="vg", name="v_g")
            self.qkv_group(blk, xlnT_g, q_g, k_g, v_g)
            oT_g = self.gp.tile([128, 4, DIM], BF16, tag="oTg", name="oT_g")
            for j in range(2):
                self.attn_sample(q_g, k_g, v_g, oT_g, 256 * j, 256,
                                 [(2 * j, 0, 128), (2 * j + 1, 0, 98)],
                                 spa_nchunks)
            self.proj_group(blk, oT_g, 4 * g)

        # spe stream (uses cls-swapped xlnT_spe; runs after all spa swaps)
        q_s = self.gp.tile([128, 4, DIM], BF16, tag="qg", name="q_s")
        k_s = self.gp.tile([128, 4, DIM], BF16, tag="kg", name="k_s")
        v_s = self.gp.tile([128, 4, NH, 65], BF16, tag="vg", name="v_s")
        self.qkv_group(blk, xlnT_spe, q_s, k_s, v_s)
        oT_s = self.gp.tile([128, 4, DIM], BF16, tag="oTg", name="oT_s")
        for s in range(SB):
            self.attn_sample(q_s, k_s, v_s, oT_s, 32 * s, 32,
                             [(s // 4, 32 * (s % 4), 31)], [(0, 32)])
        self.proj_group(blk, oT_s, NT_SPA)

        # cls fix.  Blanket residual gave X_cls = X_old + z_self.  Want
        # X_cls = 2*X_old + z_other, where z_other = xcB_other - xc0_other.
        xcB_x = self.bp.tile([16, DIM], F32, tag="xcBx", name="xcB_x")
        xcB_y = self.bp.tile([16, DIM], F32, tag="xcBy", name="xcB_y")
        self.gather_cls(xcB_x, xcB_y)
        new_x = self.bp.tile([16, DIM], F32, tag="newx", name="new_x")
        new_y = self.bp.tile([16, DIM], F32, tag="newy", name="new_y")
        nc.vector.tensor_scalar(out=new_x, in0=xc0_x, scalar1=2.0,
                                scalar2=None, op0=ALU.mult)
        nc.vector.tensor_tensor(out=new_x, in0=new_x, in1=xcB_y, op=ALU.add)
        nc.vector.tensor_tensor(out=new_x, in0=new_x, in1=xc0_y,
                                op=ALU.subtract)
        nc.vector.tensor_scalar(out=new_y, in0=xc0_y, scalar1=2.0,
                                scalar2=None, op0=ALU.mult)
        nc.vector.tensor_tensor(out=new_y, in0=new_y, in1=xcB_x, op=ALU.add)
        nc.vector.tensor_tensor(out=new_y, in0=new_y, in1=xc0_x,
                                op=ALU.subtract)
        self.scatter_cls(new_x, new_y)

        # ---- MLP ----
        wfc1, wfc2 = self.wblk[f"wfc1{blk}"], self.wblk[f"wfc2{blk}"]
        aff2 = self.bias["n2_wb"][blk] if cfg["use_n2"] else None
        stats2 = self.bp.tile([128, NT, 2], F32, tag="stats2", name="stats2")
        self.ln_stats(stats2, 0, NT)
        fc1_b = self.bias.get("fc1_b")
        for r in range(9):
            x2T = self.gp.tile([128, 4, DIM], BF16, tag="xlnT", name="x2T")
            for lc in range(4):
                self.ln_apply_T(stats2, 4 * r + lc, x2T, 128 * lc, aff2)
            h1T = self.gp.tile([128, 4, DIM], BF16, tag="oTg", name="h1T")
            for m in range(4):
                psm = self.ps_mm()
                for kc in range(4):
                    nc.tensor.matmul(psm, wfc1[:, kc, 128 * m:128 * (m + 1)],
                                     x2T[:, kc, :], start=(kc == 0),
                                     stop=(kc == 3))
                b1 = fc1_b[blk][:, m:m + 1] if fc1_b is not None else 0.0
                nc.scalar.activation(out=h1T[:, m, :], in_=psm, func=AF.Gelu,
                                     bias=b1, scale=1.0)
            for cc in range(4):
                psm = self.ps_mm()
                for kc in range(4):
                    nc.tensor.matmul(psm, h1T[:, kc, 128 * cc:128 * (cc + 1)],
                                     wfc2[:, kc, :], start=(kc == 0),
                                     stop=(kc == 3))
                if cfg["use_fc2_b"]:
                    nc.vector.tensor_tensor(
                        out=psm, in0=psm,
                        in1=self.bias["fc2_b"][blk:blk + 1, :].to_broadcast(
                            (1, DIM)), op=ALU.add)
                dt = 4 * r + cc
                nc.vector.tensor_tensor(out=X[:, dt, :], in0=X[:, dt, :],
                                        in1=psm, op=ALU.add)

    # ------------------------------------------------------------ head
    def head(self):
        nc, X, cfg = self.nc, self.X, self.cfg
        xcf = [self.bp.tile([16, DIM], F32, tag=f"xcf{h}", name=f"xcf{h}")
               for h in range(2)]
        self.gather_cls(xcf[0], xcf[1])
        clsn = [self.bp.tile([16, DIM], BF16, tag=f"clsn{h}", name=f"clsn{h}")
                for h in range(2)]
        for half in range(2):
            st = self.sp.tile([16, 6], F32, tag="lnstf", name="stf")
            mv = self.sp.tile([16, 2], F32, tag="fmv", name="mvf")
            nc.vector.bn_stats(out=st, in_=xcf[half])
            nc.vector.bn_aggr(out=mv, in_=st)
            nc.scalar.activation(out=mv[:, 1:2], in_=mv[:, 1:2], func=AF.Ln,
                                 bias=self.eps[0:16], scale=1.0)
            nc.scalar.activation(out=mv[:, 1:2], in_=mv[:, 1:2], func=AF.Exp,
                                 scale=-0.5)
            nc.vector.tensor_scalar(out=clsn[half], in0=xcf[half],
                                    scalar1=mv[:, 0:1], scalar2=mv[:, 1:2],
                                    op0=ALU.subtract, op1=ALU.mult)
            if cfg["use_nf"]:
                nc.vector.tensor_tensor(
                    out=clsn[half], in0=clsn[half],
                    in1=self.bias["nf_w"][half:half + 1, :].to_broadcast((1, DIM)),
                    op=ALU.mult)
                nc.vector.tensor_tensor(
                    out=clsn[half], in0=clsn[half],
                    in1=self.bias["nf_b"][half:half + 1, :].to_broadcast((1, DIM)),
                    op=ALU.add)
        clsT = self.bp.tile([128, 8, NCLS], BF16, tag="clsT", name="clsT")
        for half in range(2):
            for e in range(4):
                pt = self.ps_tr()
                nc.tensor.transpose(pt[:, :16],
                                    clsn[half][:, 128 * e:128 * (e + 1)],
                                    self.ident[:16, :16])
                nc.any.tensor_copy(out=clsT[:, 4 * half + e, :], in_=pt[:, :16])
        psh = self.ps.tile([16, NCLS], F32, tag="mm", bufs=2, name="psh")
        for dc in range(8):
            nc.tensor.matmul(psh, clsT[:, dc, :], self.whead[:, dc, :],
                             start=(dc == 0), stop=(dc == 7))
        out_sb = self.bp.tile([16, NCLS], F32, tag="out_sb", name="out_sb")
        if cfg["use_head_b"]:
            nc.vector.tensor_tensor(
                out=out_sb, in0=psh,
                in1=self.bias["head_b"][0:1, :].to_broadcast((1, NCLS)),
                op=ALU.add)
        else:
            nc.vector.tensor_copy(out=out_sb, in_=psh)
        nc.sync.dma_start(out=self.out_p[:], in_=out_sb)


# ---------------------------------------------------------------- entry
def kernel(**inputs):
    w = prepare_weights(inputs)
    nc = build_program(w)
    in_maps = make_in_maps(inputs, w)
    res = run_bass_kernel_spmd(nc, in_maps, list(range(NCORES)))
    out = np.concatenate([np.asarray(r["out"], np.float32)
                          for r in res.results], axis=0)
    return out


def run_traced(inputs):
    """For test.py: returns (out, BassKernelResults with exec_time_ns)."""
    w = prepare_weights(inputs)
    nc = build_program(w)
    in_maps = make_in_maps(inputs, w)
    res = run_bass_kernel_spmd(nc, in_maps, list(range(NCORES)), trace=True)
    out = np.concatenate([np.asarray(r["out"], np.float32)
                          for r in res.results], axis=0)
    return out, res



# revision 63
# speedup vs baseline: 1.4585x; 1.4585x over previous
"""Trainium2 Bass kernel for nn_CASST (dense transformer, CTMF blocks).

Self-contained: builds the Bass program from the concrete numpy inputs,
shards batch B=128 across 8 NeuronCores (16 samples each), runs SPMD,
gathers the full [128, 16] output.

Per-core layout:
  X [128, 36, 512] f32 token-major residual: tiles 0..31 = spatial stream
  (16 samples x 256 padded rows, 226 valid: cls at row 0, patches 1..225),
  tiles 32..35 = spectral stream (16 samples x 32 padded rows, 31 valid).
  Matmul operands bf16; PSUM accumulation f32.
  Attention: scores computed transposed (keys on partitions) so softmax
  needs no transpose of the attention matrix; the row-sum comes free from
  an extra all-ones column appended to V, and the normalization is fused
  into the PSUM->SBUF copy of the per-head output.
"""
import os
import sys

sys.path.insert(0, "/opt/trn_rl_repo")

_SKIP = set(os.environ.get("K_SKIP", "").split(","))  # debug bisect only

import numpy as np
import ml_dtypes

import concourse.bass as bass
import concourse.tile as tile
from concourse import bacc
from concourse import mybir
from concourse.masks import make_identity
from concourse.bass_utils import run_bass_kernel_spmd

F32 = mybir.dt.float32
BF16 = mybir.dt.bfloat16
_F8DT = mybir.dt.float8e4
_DRPM = mybir.MatmulPerfMode.DoubleRow
AF = mybir.ActivationFunctionType
ALU = mybir.AluOpType

# Per-GEMM fp8/DoubleRow config.  K_BF16=1 disables all; K_F8=qkv,av,...
# selects a subset (default: all on).
_ALL_G = ("qkv", "av", "proj", "fc1", "fc2")
if os.environ.get("K_BF16") == "1":
    _F8G = set()
elif os.environ.get("K_F8") is not None:
    _F8G = set(os.environ["K_F8"].split(",")) & set(_ALL_G)
else:
    # default: all bf16 — single-pass e4m3 on the big GEMMs costs ~3% output
    # noise per GEMM, which blows the 2e-2 correctness budget (measured)
    _F8G = set()
_BF16_MODE = not _F8G


def _dt8(g):
    return _F8DT if g in _F8G else BF16


def _kcs(g):
    return ((0, 2), 2, _DRPM) if g in _F8G else ((0, 1, 2, 3), 1, None)

# Global residual scale: the residual stream X carries SCL*x.  LayerNorm is
# scale-invariant, so LN outputs are unscaled; fp8 weights are stored as
# SCL*w (lifting ~0.02-magnitude weights out of e4m3's subnormal range) and
# every 1/SCL folds into an op that already exists (q's softmax scale, the
# attention rowsum column, fc1's gelu pre-scale, the residual adds).
SCL = 32.0

B, BANDS, HW, DIM, NH, NCLS = 128, 30, 15, 512, 8, 16
NCORES = 8
SB = B // NCORES          # 16 samples per core
NPATCH = HW * HW          # 225
SPA_PAD, SPE_PAD = 256, 32
NT_SPA = SB * SPA_PAD // 128   # 32
NT_SPE = SB * SPE_PAD // 128   # 4
NT = NT_SPA + NT_SPE           # 36
EPS = 1e-5
HD = DIM // NH            # 64


def _bf(x):
    return np.asarray(x, dtype=np.float32).astype(ml_dtypes.bfloat16)


def _f8(x, g):
    if g not in _F8G:
        return _bf(x)
    return np.asarray(x, dtype=np.float32).astype(ml_dtypes.float8_e4m3)


def _pack_kT(w_T, mdim):
    """[512, M] (contraction rows) -> [128, 4, M] (partition, k-chunk, M)."""
    k = w_T.shape[0]
    return np.ascontiguousarray(w_T.reshape(k // 128, 128, mdim).transpose(1, 0, 2))


def _ident(a, b):
    return bool(np.all(np.asarray(a) == 1) and np.all(np.asarray(b) == 0))


def prepare_weights(inp):
    w = {}
    # spatial conv + BN fold
    s_h = inp["bn_h_g"] / np.sqrt(inp["bn_h_v"] + EPS)
    b_h = inp["conv_h_b"] * s_h + inp["bn_h_b"] - inp["bn_h_m"] * s_h
    w_h = np.asarray(inp["conv_h_w"]) * np.asarray(s_h)[:, None, None, None]
    # K-tile ky: row 32*kx + b holds tap (ky,kx) band b; tile0 row 96 = bias
    w_spa = np.zeros((3, 128, DIM), np.float32)
    for ky in range(3):
        for kx in range(3):
            w_spa[ky, 32 * kx:32 * kx + 30, :] = np.asarray(w_h)[:, :, ky, kx].T
    w_spa[0, 96, :] = np.asarray(b_h)
    w["w_spa"] = _bf(w_spa * SCL)

    # spectral conv + BN fold
    s_c = inp["cnn_bn_g"] / np.sqrt(inp["cnn_bn_v"] + EPS)
    b_c = inp["cnn_conv_b"] * s_c + inp["cnn_bn_b"] - inp["cnn_bn_m"] * s_c
    w_c = np.asarray(inp["cnn_conv_w"]) * np.asarray(s_c)[:, None, None, None]
    # row 32*kx + ky holds tap (ky,kx) so border memsets stay 32-aligned
    w_cnn = np.zeros((68, 128), np.float32)
    for ky in range(3):
        for kx in range(3):
            w_cnn[32 * kx + ky, :] = w_c[:, 0, ky, kx]
    w_cnn[67, :] = b_c          # bias row, matched by an all-ones im2col row
    w["w_cnn"] = _bf(w_cnn)
    w["w_fc"] = _bf(np.asarray(inp["cnn_fc_w"]).T / NPATCH * SCL)  # mean folded
    w["fc_b"] = np.asarray(inp["cnn_fc_b"], np.float32) * SCL

    for i in range(2):
        qkv = np.asarray(inp["blk_qkv_w"][i])
        wq, wk, wv = qkv[:DIM], qkv[DIM:2 * DIM], qkv[2 * DIM:]
        w[f"wqk{i}"] = _f8(_pack_kT(np.concatenate([wq.T, wk.T], 1), 2 * DIM) * SCL, "qkv")
        w[f"wv{i}"] = _f8(_pack_kT(wv.T, DIM) * SCL, "qkv")
        w[f"wproj{i}"] = _f8(_pack_kT(np.asarray(inp["blk_proj_w"][i]).T, DIM) * SCL, "proj")
        w[f"wfc1{i}"] = _f8(_pack_kT(np.asarray(inp["blk_fc1_w"][i]).T, DIM) * SCL, "fc1")
        w[f"wfc2{i}"] = _f8(_pack_kT(np.asarray(inp["blk_fc2_w"][i]).T, DIM) * SCL, "fc2")
    w["whead"] = _bf(_pack_kT(np.asarray(inp["head_w"]).T, NCLS))

    for k in ("blk_qkv_b", "blk_proj_b", "blk_fc1_b", "blk_fc2_b", "head_b",
              "blk_n1_w", "blk_n1_b", "blk_n2_w", "blk_n2_b",
              "norm1_w", "norm1_b", "norm2_w", "norm2_b"):
        w[k] = np.asarray(inp[k], np.float32)
    # residual-scale folds: q/k psums carry SCL (bias must match); proj/fc2
    # psums feed the SCL-scaled residual directly.  fc1_b stays raw (applied
    # on the unscaled gelu input via scale=1/SCL).
    w["blk_qkv_b"] = w["blk_qkv_b"] * SCL
    w["blk_proj_b"] = w["blk_proj_b"] * SCL
    w["blk_fc2_b"] = w["blk_fc2_b"] * SCL

    # pos_spa[p, c] = positional embedding for the token that conv-psum row p
    # of chunk c produces: chunk0 row p -> token p+1, chunk1 row p -> token 128+p
    pos_shift = np.zeros((128, 2, DIM), np.float32)
    spa_pos = np.asarray(inp["spa_pos"])[0]           # [226, 512]
    pos_shift[0:127, 0] = spa_pos[1:128]
    pos_shift[0:98, 1] = spa_pos[128:226]
    w["pos_spa"] = pos_shift * SCL
    pos_spe = np.zeros((SPE_PAD, DIM), np.float32)
    pos_spe[1:1 + BANDS] = np.asarray(inp["spe_pos"])[0, 1:1 + BANDS]
    w["pos_spe"] = np.ascontiguousarray(np.tile(pos_spe, (4, 1))) * SCL
    cls2 = np.zeros((2, DIM), np.float32)
    cls2[0] = np.asarray(inp["spa_cls"])[0, 0] + np.asarray(inp["spa_pos"])[0, 0]
    cls2[1] = np.asarray(inp["spe_cls"])[0, 0] + np.asarray(inp["spe_pos"])[0, 0]
    w["cls2"] = cls2 * SCL

    w["cfg"] = dict(
        use_qkv_b=bool(np.any(w["blk_qkv_b"] != 0)),
        use_proj_b=bool(np.any(w["blk_proj_b"] != 0)),
        use_fc1_b=bool(np.any(w["blk_fc1_b"] != 0)),
        use_fc2_b=bool(np.any(w["blk_fc2_b"] != 0)),
        use_fc_b=bool(np.any(w["fc_b"] != 0)),
        use_head_b=bool(np.any(w["head_b"] != 0)),
        use_n1=not all(_ident(w["blk_n1_w"][i], w["blk_n1_b"][i]) for i in range(2)),
        use_n2=not all(_ident(w["blk_n2_w"][i], w["blk_n2_b"][i]) for i in range(2)),
        use_nf=not (_ident(w["norm1_w"], w["norm1_b"])
                    and _ident(w["norm2_w"], w["norm2_b"])),
    )
    return w


def _im2cols(xc):
    """Host im2col for one core's x shard [SB, 30, 15, 15] (f32)."""
    xp = np.pad(xc, ((0, 0), (0, 0), (1, 1), (1, 1)))
    im_spa = np.zeros((3, 128, SB * NPATCH), np.float32)
    im_spe = np.zeros((68, SB * BANDS * NPATCH), np.float32)
    for ky in range(3):
        for kx in range(3):
            win = xp[:, :, ky:ky + HW, kx:kx + HW]          # [SB,30,15,15]
            im_spa[ky, 32 * kx:32 * kx + 30, :] = (
                win.transpose(1, 0, 2, 3).reshape(BANDS, -1))
            im_spe[32 * kx + ky, :] = win.reshape(-1)
    im_spa[0, 96, :] = 1.0
    im_spe[67, :] = 1.0
    return _bf(im_spa), _bf(im_spe)


def make_in_maps(inputs, w):
    x = np.asarray(inputs["x"], np.float32)[:, 0]   # [128, 30, 15, 15]
    cfg = w["cfg"]
    base = {k: w[k] for k in
            ("w_spa", "w_cnn", "w_fc", "whead", "pos_spa",
             "pos_spe", "cls2")}
    for i in range(2):
        for nm in ("wqk", "wv", "wproj", "wfc1", "wfc2"):
            base[nm + str(i)] = w[nm + str(i)]
    if cfg["use_qkv_b"]:
        base["qkv_b"] = np.ascontiguousarray(
            w["blk_qkv_b"].reshape(2, 12, 128).transpose(0, 2, 1))
    if cfg["use_proj_b"]:
        base["proj_b"] = w["blk_proj_b"]
    if cfg["use_fc1_b"]:
        base["fc1_b"] = np.ascontiguousarray(
            w["blk_fc1_b"].reshape(2, 4, 128).transpose(0, 2, 1))
    if cfg["use_fc2_b"]:
        base["fc2_b"] = w["blk_fc2_b"]
    if cfg["use_fc_b"]:
        base["fc_b"] = w["fc_b"].reshape(1, DIM)
    if cfg["use_head_b"]:
        base["head_b"] = w["head_b"].reshape(1, NCLS)
    if cfg["use_n1"]:
        base["n1_wb"] = np.ascontiguousarray(
            np.stack([w["blk_n1_w"], w["blk_n1_b"]], axis=1))
    if cfg["use_n2"]:
        base["n2_wb"] = np.ascontiguousarray(
            np.stack([w["blk_n2_w"], w["blk_n2_b"]], axis=1))
    if cfg["use_nf"]:
        base["nf_w"] = np.stack([w["norm1_w"], w["norm2_w"]])
        base["nf_b"] = np.stack([w["norm1_b"], w["norm2_b"]])
    maps = []
    for c in range(NCORES):
        m = dict(base)
        m["im_spa"], m["im_spe"] = _im2cols(x[c * SB:(c + 1) * SB])
        maps.append(m)
    return maps


# =====================================================================
def build_program(w, debug_stage=None):
    cfg = w["cfg"]
    nc = bacc.Bacc(None)
    P = {}

    def dparam(name, shape, dt):
        P[name] = nc.declare_dram_parameter(name, list(shape), dt, isOutput=False)

    dparam("im_spa", (3, 128, SB * NPATCH), BF16)
    dparam("im_spe", (68, SB * BANDS * NPATCH), BF16)
    dparam("w_spa", (3, 128, DIM), BF16)
    dparam("w_cnn", (68, 128), BF16)
    dparam("w_fc", (128, DIM), BF16)
    _wg = {"wqk": "qkv", "wv": "qkv", "wproj": "proj", "wfc1": "fc1",
           "wfc2": "fc2"}
    for i in range(2):
        dparam(f"wqk{i}", (128, 4, 2 * DIM), _dt8("qkv"))
        for nm in ("wv", "wproj", "wfc1", "wfc2"):
            dparam(f"{nm}{i}", (128, 4, DIM), _dt8(_wg[nm]))
    dparam("whead", (128, 8, NCLS), BF16)
    dparam("pos_spa", (128, 2, DIM), F32)
    dparam("pos_spe", (128, DIM), F32)
    dparam("cls2", (2, DIM), F32)
    if cfg["use_qkv_b"]:
        dparam("qkv_b", (2, 128, 12), F32)
    if cfg["use_proj_b"]:
        dparam("proj_b", (2, DIM), F32)
    if cfg["use_fc1_b"]:
        dparam("fc1_b", (2, 128, 4), F32)
    if cfg["use_fc2_b"]:
        dparam("fc2_b", (2, DIM), F32)
    if cfg["use_fc_b"]:
        dparam("fc_b", (1, DIM), F32)
    if cfg["use_head_b"]:
        dparam("head_b", (1, NCLS), F32)
    if cfg["use_n1"]:
        dparam("n1_wb", (2, 2, DIM), F32)
    if cfg["use_n2"]:
        dparam("n2_wb", (2, 2, DIM), F32)
    if cfg["use_nf"]:
        dparam("nf_w", (2, DIM), F32)
        dparam("nf_b", (2, DIM), F32)
    out_p = nc.declare_dram_parameter("out", [SB, NCLS], F32, isOutput=True)
    dbg_p = None
    if debug_stage is not None:
        dbg_p = nc.declare_dram_parameter("dbgX", [128, NT, DIM], BF16,
                                          isOutput=True)

    with tile.TileContext(nc) as tc:
        Kernel(tc, P, out_p, cfg, debug_stage, dbg_p).build()
    nc.finalize()   # Bacc: runs wait-splitting legalization + reg alloc
    return nc


class Kernel:
    def __init__(self, tc, P, out_p, cfg, debug_stage=None, dbg_p=None):
        self.tc, self.nc, self.P, self.out_p, self.cfg = tc, tc.nc, P, out_p, cfg
        self.debug_stage, self.dbg_p = debug_stage, dbg_p

    def dbg_dump(self, stage):
        if self.debug_stage == stage:
            self.nc.sync.dma_start(out=self.dbg_p[:], in_=self.X)

    def dbg_dump_tile(self, stage, ap):
        """Dump an arbitrary [128, N] SBUF tile into dbgX[:, 0, :N]."""
        if self.debug_stage == stage:
            n = ap.shape[-1]
            self.nc.sync.dma_start(out=self.dbg_p[:, 0, 0:n], in_=ap)

    def build(self):
        tc, nc, P = self.tc, self.nc, self.P
        with tc.tile_pool(name="const", bufs=1) as cp:
            self.cp = cp
            X = cp.tile([128, NT, DIM], BF16, name="X")
            self.X = X
            # only spatial pad rows (98:128 of odd tiles) are never written
            # by the conv stage; 96:128 is the 32-aligned window (96-97 get
            # rewritten by the conv)
            nc.vector.memset(X[96:128, 1:NT_SPA:2, :], 0.0)
            ident_bf = cp.tile([128, 128], BF16, name="ident_bf")
            make_identity(nc, ident_bf)
            self.ident = ident_bf
            eps_sb = cp.tile([128, 1], F32, name="eps_sb")
            nc.vector.memset(eps_sb, EPS)
            self.eps = eps_sb

            def load(name, shape, dt, src=None):
                t = cp.tile(list(shape), dt, name="sb_" + name)
                nc.sync.dma_start(out=t, in_=src if src is not None else P[name][:])
                return t

            self.w_spa = load("w_spa", (128, 3, DIM), BF16,
                              P["w_spa"][:].rearrange("a p m -> p a m"))
            self.w_cnn = load("w_cnn", (68, 128), BF16)
            self.w_fc = load("w_fc", (128, DIM), BF16)
            self.whead = load("whead", (128, 8, NCLS), BF16)
            self.pos_spa = load("pos_spa", (128, 2, DIM), F32)
            self.pos_spe = load("pos_spe", (128, DIM), F32)
            self.cls2 = load("cls2", (1, 2, DIM), F32,
                             P["cls2"][:].rearrange("a d -> 1 a d") if False
                             else P["cls2"][None, :, :])
            self.bias = {}
            for k, shp in (("qkv_b", (2, 128, 12)), ("proj_b", (2, DIM)),
                           ("fc1_b", (2, 128, 4)), ("fc2_b", (2, DIM)),
                           ("fc_b", (1, DIM)), ("head_b", (1, NCLS)),
                           ("n1_wb", (2, 2, DIM)), ("n2_wb", (2, 2, DIM)),
                           ("nf_w", (2, DIM)), ("nf_b", (2, DIM))):
                if k in P:
                    self.bias[k] = load(k, shp, F32)

            self.pool_sb = cp.tile([128, SB * SPE_PAD], F32, name="pool_sb")
            nc.vector.memset(self.pool_sb, 0.0)

            with tc.tile_pool(name="convp", bufs=1) as cvp, \
                 tc.tile_pool(name="convtmp", bufs=3) as cvt, \
                 tc.tile_pool(name="convps", bufs=1, space="PSUM") as cps:
                self.conv_stage(cvp, cvt, cps)

            self.dbg_dump(0)
            with tc.tile_pool(name="wblk", bufs=1) as wp, \
                 tc.tile_pool(name="blk", bufs=1) as bp, \
                 tc.tile_pool(name="grp", bufs=2) as gp, \
                 tc.tile_pool(name="attn", bufs=4) as ap, \
                 tc.tile_pool(name="small", bufs=4) as sp, \
                 tc.tile_pool(name="ps", bufs=1, space="PSUM") as ps:
                self.wp, self.bp, self.gp, self.ap, self.sp, self.ps = \
                    wp, bp, gp, ap, sp, ps
                for i in range(2):
                    if self.debug_stage is not None and self.debug_stage <= i:
                        break
                    self.block(i)
                    self.dbg_dump(i + 1)
                self.head()

    # psum helpers: one pool, explicit per-tag bufs (total <= 8 banks)
    def ps_mm(self):
        return self.ps.tile([128, DIM], F32, tag="mm", bufs=2, name="ps_mm")

    def ps_tr4(self):
        # full-bank bf16 tile (1024 cols = 2KB) so each ring slot owns a
        # whole PSUM bank and transpose outputs land at bank offset 0
        return self.ps.tile([128, 1024], BF16, tag="tr4", bufs=2,
                            name="ps_tr4")

    def ps_sT(self):
        # one head's scores (both key chunks merged) -> one 512-wide exp.
        # bufs=2 keeps the PE streaming: head h+1's scores run while head
        # h's exp drains (single-buffering here measurably cold-clocks PE).
        return self.ps.tile([128, DIM], F32, tag="sT", bufs=2, name="ps_sT")

    def ps_sTs(self):
        # spe scores: all 8 heads of one 31-key sample; shares the sT tag
        # (PSUM is fully subscribed at 8 banks)
        return self.ps.tile([128, NH, 32], F32, tag="sT", bufs=2,
                            name="ps_sTs")

    def ps_o(self):
        return self.ps.tile([128, 4 * 65], F32, tag="ops", bufs=2, name="ps_o")

    # ------------------------------------------------------------ conv
    def conv_stage(self, cvp, cvt, cps):
        nc, X = self.nc, self.X
        # host-built im2cols, plain DMA loads
        im = [cvp.tile([97, SB, NPATCH], BF16, name=f"im_spa{k}")
              for k in range(3)]
        for k in range(3):
            nc.sync.dma_start(
                out=im[k],
                in_=self.P["im_spa"][k, 0:97].rearrange(
                    "k (s p) -> k s p", s=SB))
        imf = im

        sc_spa = self.nc.named_scope("conv_spa")
        sc_spa.__enter__()
        for s in range(SB):
            for ci, (p0, p1) in enumerate(((0, 127), (127, 225))):
                m = p1 - p0
                psm = cps.tile([128, DIM], F32, tag="spaps", bufs=3,
                               name="psm_spa")
                for k in range(3):
                    kv = 97 if k == 0 else 94
                    nc.tensor.matmul(psm[:m], imf[k][:kv, s, p0:p1],
                                     self.w_spa[:kv, k, :],
                                     start=(k == 0), stop=(k == 2))
                tmp = cvt.tile([128, DIM], BF16, tag="spatmp", name="tmp_spa")
                nc.scalar.activation(out=tmp[:m], in_=psm[:m], func=AF.Relu)
                nc.vector.tensor_tensor(out=tmp[:m], in0=tmp[:m],
                                        in1=self.pos_spa[:m, ci, :],
                                        op=ALU.add)
                if ci == 0:
                    # token rows 1..127 of tile 2s: partition base 1 is not
                    # engine-addressable -> bounce through DMA
                    nc.sync.dma_start(out=X[1:128, 2 * s, :], in_=tmp[:m])
                else:
                    nc.vector.tensor_copy(out=X[0:98, 2 * s + 1, :],
                                          in_=tmp[:m])

        sc_spa.__exit__(None, None, None)
        sc_spe = self.nc.named_scope("conv_spe")
        sc_spe.__enter__()
        # spectral conv, computed TRANSPOSED ([pos, chan]; bias folds into
        # the all-ones K-row 67) so the 225-position sum-pool becomes two
        # nearly-free N=1 ones-matmuls on the PE instead of an engine
        # reduce.  Evacuation is ONE relu per 2-instance pair, alternating
        # ACT/DVE.
        ones_bf = cvp.tile([128, 1], BF16, name="ones_bf")
        nc.vector.memset(ones_bf, 1.0)
        im2 = [cvp.tile([68, 15, NPATCH], BF16, name=f"im_spe{k}")
               for k in range(3)]
        im_spe_p = self.P["im_spe"][:].rearrange("k (i p) -> k i p", p=NPATCH)
        pchunk = ((0, 128), (128, 97))
        for cc in range(SB * 2):
            s, h2 = cc // 2, cc % 2
            t = im2[cc % 3]
            i_base = 30 * s + 15 * h2
            nc.sync.dma_start(out=t, in_=im_spe_p[:, i_base:i_base + 15, :])
            pool_ps = cps.tile([128, 16], F32, tag="poolps", bufs=1,
                               name="pool_ps")
            for g in range(8):
                i0, i1 = 2 * g, min(2 * g + 2, 15)
                ni = i1 - i0
                psm = cps.tile([128, 4, 128], F32, tag="speps", bufs=2,
                               name="psm_spe")
                for li in range(ni):
                    for c, (q0, qn) in enumerate(pchunk):
                        nc.tensor.matmul(
                            psm[:qn, 2 * li + c, :],
                            t[:, i0 + li, q0:q0 + qn],
                            self.w_cnn, start=True, stop=True)
                relu_t = cvt.tile([128, 4, 128], BF16, tag="relu_t",
                                  name="relu_t")
                if (cc * 8 + g) % 2 == 0:
                    nc.scalar.activation(
                        out=relu_t[:, :2 * ni, :].rearrange("p a b -> p (a b)"),
                        in_=psm[:, :2 * ni, :].rearrange("p a b -> p (a b)"),
                        func=AF.Relu)
                else:
                    nc.vector.tensor_scalar(
                        out=relu_t[:, :2 * ni, :].rearrange("p a b -> p (a b)"),
                        in0=psm[:, :2 * ni, :].rearrange("p a b -> p (a b)"),
                        scalar1=0.0, scalar2=None, op0=ALU.max)
                for li in range(ni):
                    for c, (q0, qn) in enumerate(pchunk):
                        nc.tensor.matmul(
                            pool_ps[:, i0 + li:i0 + li + 1],
                            relu_t[:qn, 2 * li + c, :],
                            ones_bf[:qn, :], start=(c == 0), stop=(c == 1),
                            skip_group_check=True)
            col0 = SPE_PAD * s + 1 + 15 * h2
            nc.vector.tensor_copy(out=self.pool_sb[:, col0:col0 + 15],
                                  in_=pool_ps[:, 0:15])

        sc_spe.__exit__(None, None, None)
        sc_fc = self.nc.named_scope("conv_fc")
        sc_fc.__enter__()
        self.dbg_dump_tile(10, self.pool_sb)
        pool_bf = cvp.tile([128, SB * SPE_PAD], BF16, name="pool_bf")
        nc.vector.tensor_copy(out=pool_bf, in_=self.pool_sb)
        for g in range(4):
            psm = cps.tile([128, DIM], F32, tag="fcps", bufs=2, name="psm_fc")
            nc.tensor.matmul(psm, pool_bf[:, 128 * g:128 * (g + 1)], self.w_fc,
                             start=True, stop=True)
            tmpf = cvt.tile([128, DIM], BF16, tag="fctmp", name="tmp_fc")
            nc.scalar.activation(out=tmpf, in_=psm, func=AF.Relu)
            if self.cfg["use_fc_b"]:
                nc.vector.tensor_tensor(
                    out=tmpf, in0=tmpf,
                    in1=self.bias["fc_b"][0:1, :].to_broadcast((1, DIM)),
                    op=ALU.add)
            nc.vector.tensor_tensor(out=X[:, NT_SPA + g, :], in0=tmpf,
                                    in1=self.pos_spe, op=ALU.add)

        sc_fc.__exit__(None, None, None)
        # cls tokens
        nc.vector.tensor_copy(
            out=X[0:1, 0:NT_SPA:2, :],
            in_=self.cls2[0:1, 0:1, :].to_broadcast((1, SB, DIM)))
        for k in range(4):
            nc.vector.tensor_copy(
                out=X[32 * k:32 * k + 1, NT_SPA:NT, :],
                in_=self.cls2[0:1, 1:2, :].to_broadcast((1, 4, DIM)))

    # ------------------------------------------------------------ layernorm
    def ln_stats(self, stats, c0, c1):
        """Stats for chunks [c0, c1); batched invstd = exp(-0.5*ln(var+eps))
        (strided over the range — per-chunk ACT ops thrashed the table set,
        a single full-tile op serialized the block start)."""
        nc, X = self.nc, self.X
        for c in range(c0, c1):
            st = self.sp.tile([128, 6], F32, tag="lnst", name="st")
            nc.vector.bn_stats(out=st, in_=X[:, c, :])
            nc.vector.bn_aggr(out=stats[:, c, :], in_=st)
        v = stats[:, c0:c1, 1:2]
        nc.scalar.activation(out=v, in_=v, func=AF.Ln, bias=self.eps,
                             scale=1.0)
        nc.scalar.activation(out=v, in_=v, func=AF.Exp, scale=-0.5)

    def ln_apply_T(self, stats, c, dst, dst_col, affine=None):
        """LN chunk c -> transpose -> dst[:, e, dst_col:+128] (bf16)."""
        nc, X = self.nc, self.X
        lno = self.sp.tile([128, DIM], BF16, tag="lno", name="lno")
        nc.vector.tensor_scalar(out=lno, in0=X[:, c, :],
                                scalar1=stats[:, c, 0:1],
                                scalar2=stats[:, c, 1:2],
                                op0=ALU.subtract, op1=ALU.mult)
        if affine is not None:
            nc.vector.tensor_tensor(out=lno, in0=lno,
                                    in1=affine[0:1, :].to_broadcast((1, DIM)),
                                    op=ALU.mult)
            nc.vector.tensor_tensor(out=lno, in0=lno,
                                    in1=affine[1:2, :].to_broadcast((1, DIM)),
                                    op=ALU.add)
        pt4 = self.ps_tr4().rearrange("p (a b) -> p a b", a=4)[:, :, 0:128]
        for e in range(4):
            nc.tensor.transpose(pt4[:, e, :], lno[:, 128 * e:128 * (e + 1)],
                                self.ident)
        nc.any.tensor_copy(out=dst[:, :, dst_col:dst_col + 128], in_=pt4)

    # ------------------------------------------------------------ block
    def qkv_group(self, blk, xlnT_g, q_g, k_g, v_g, spa=True):
        # fp8 DoubleRow: each matmul contracts two 128-row K-chunks.
        # psums carry SCL (weights stored as SCL*w); q folds 1/SCL^2 into the
        # softmax scale (k keeps its SCL), v keeps SCL which the rowsum
        # column (memset to SCL) cancels at normalization.
        nc = self.nc
        wqk, wv = self.wblk[f"wqk{blk}"], self.wblk[f"wv{blk}"]
        qkv_b = self.bias.get("qkv_b")
        qscale = float(HD) ** -0.5 / (SCL * SCL)
        kcs, kw, pm = _kcs("qkv")
        for m in range(8):
            psm = self.ps_mm()
            for kc in kcs:
                nc.tensor.matmul(psm, wqk[:, kc:kc + kw, 128 * m:128 * (m + 1)],
                                 xlnT_g[:, kc:kc + kw, :], start=(kc == 0),
                                 stop=(kc == kcs[-1]), perf_mode=pm)
            dst = q_g[:, m, :] if m < 4 else k_g[:, m - 4, :]
            if m < 4:
                if qkv_b is not None:
                    nc.vector.tensor_scalar(out=dst, in0=psm,
                                            scalar1=qkv_b[blk][:, m:m + 1],
                                            scalar2=qscale,
                                            op0=ALU.add, op1=ALU.mult)
                else:
                    nc.vector.tensor_scalar(out=dst, in0=psm,
                                            scalar1=qscale,
                                            scalar2=None, op0=ALU.mult)
            else:
                if qkv_b is not None:
                    nc.gpsimd.tensor_scalar(out=dst, in0=psm,
                                            scalar1=qkv_b[blk][:, m:m + 1],
                                            scalar2=None, op0=ALU.add)
                else:
                    nc.gpsimd.tensor_copy(out=dst, in_=psm)
        for t in range(4):
            psm = self.ps_mm()
            for kc in kcs:
                nc.tensor.matmul(psm, xlnT_g[:, kc:kc + kw, 128 * t:128 * (t + 1)],
                                 wv[:, kc:kc + kw, :], start=(kc == 0),
                                 stop=(kc == kcs[-1]), perf_mode=pm)
            nc.gpsimd.tensor_copy(out=v_g[:, t, :, 0:64], in_=psm)
        if spa:
            # DoubleRow AV sees all 128 rows of the odd (98-valid) chunks.
            # X pad rows are kept exactly zero (residual adds are masked), so
            # v/k for pad keys are zero; only the rowsum column needs masking.
            # Partition bases must be 32-aligned: zero [96:128] then set
            # [0:98], which rewrites rows 96-97.
            nc.vector.memset(v_g[96:128, 1:4:2, :, 64:65], 0.0)
            nc.vector.memset(v_g[0:98, 1:4:2, :, 64:65], SCL)
            nc.vector.memset(v_g[:, 0:4:2, :, 64:65], SCL)
        else:
            nc.vector.memset(v_g[:, :, :, 64:65], SCL)

    def attn_spa(self, q_g, k_g, v_g, j):
        """Compute phase for one spatial sample (tiles 2j, 2j+1; queries at
        columns 256j..+255): scores/exp/AV per head and the rowsum
        normalization into a token-major o_sb, which attn_spa_evac then
        transposes out.  Split so the caller can overlap the next sample's
        scores with this sample's evacuation."""
        nc = self.nc
        n0, mt0 = 256 * j, 2 * j
        o_sb = self.ap.tile([128, 2, DIM], BF16, tag="osb", name="o_sb")
        nchunks = ((0, 128), (128, 128))
        for og in range(2):
            o_ps = [self.ps_o() for _ in nchunks]
            def eject(hh, aTx):
                # AV for a head whose exp has already been issued
                h = 4 * og + hh
                for ni, (nc0, nr) in enumerate(nchunks):
                    if "av" in _F8G:
                        nc.tensor.matmul(
                            o_ps[ni][:nr, 65 * hh:65 * hh + 65],
                            aTx[:, 0:2, nc0:nc0 + nr],
                            v_g[:, mt0:mt0 + 2, h, :],
                            start=True, stop=True, perf_mode=_DRPM)
                    else:
                        for mi in range(2):
                            nc.tensor.matmul(
                                o_ps[ni][:nr, 65 * hh:65 * hh + 65],
                                aTx[:, mi, nc0:nc0 + nr],
                                v_g[:, mt0 + mi, h, :],
                                start=(mi == 0), stop=(mi == 1))

            # Software pipeline: emit head hh+1's scores BEFORE head hh's AV
            # so the PE queue never head-of-line blocks on the exp.
            pend = None
            for hh in range(4):
                h = 4 * og + hh
                hp, hc = 64 * (h % 2), h // 2
                aTx = self.ap.tile([128, 2, 256], _dt8("av"), tag="aTx",
                                   name="aTx")
                pss = self.ps_sT()
                for mi in range(2):
                    nc.tensor.matmul(
                        pss[:, 256 * mi:256 * mi + 256],
                        k_g[hp:hp + 64, hc,
                            128 * (mt0 + mi):128 * (mt0 + mi + 1)],
                        q_g[hp:hp + 64, hc, n0:n0 + 256],
                        start=True, stop=True)
                if pend is not None:
                    eject(*pend)
                nc.scalar.activation(
                    out=aTx.rearrange("p a b -> p (a b)"), in_=pss,
                    func=AF.Exp)
                pend = (hh, aTx)
            eject(*pend)
            for ni, (nc0, nr) in enumerate(nchunks):
                rinv = self.sp.tile([128, 8], F32, tag="rinv", name="rinv")
                nc.vector.reciprocal(out=rinv[:nr, 4 * og:4 * og + 4],
                                     in_=o_ps[ni][:nr, 64:260:65])
                # one op for all 4 heads: broadcast 1/rowsum over head dim
                src = o_ps[ni][:nr, :260].rearrange(
                    "p (h e) -> p h e", e=65)[:, :, 0:64]
                dst = o_sb[:nr, ni, 256 * og:256 * og + 256].rearrange(
                    "p (h e) -> p h e", e=64)
                nc.gpsimd.tensor_tensor(
                    out=dst, in0=src,
                    in1=rinv[:nr, 4 * og:4 * og + 4, None].to_broadcast(
                        (nr, 4, 64)),
                    op=ALU.mult)
        return o_sb

    def attn_spa_evac(self, o_sb, oT_g, j):
        # transpose o (token-major) -> oT_g feature-major columns
        nc = self.nc
        n0 = 256 * j
        for ni in range(2):
            pt4 = self.ps_tr4().rearrange("p (a b) -> p a b", a=4)[:, :, 0:128]
            for e in range(4):
                nc.tensor.transpose(pt4[:, e, :],
                                    o_sb[:, ni, 128 * e:128 * (e + 1)],
                                    self.ident)
            nc.any.tensor_copy(
                out=oT_g[:, :, n0 + 128 * ni:n0 + 128 * ni + 128], in_=pt4)

    def attn_spe_scores(self, q_s, k_s, s):
        """Scores+exp for one spectral sample (31 keys at rows 32(s%4)..+30
        of tile s//4, queries at cols 32s..+31).  Split from the AV half so
        the caller can overlap sample s+1's exps with sample s's AVs."""
        nc = self.nc
        mt, mb = s // 4, 32 * (s % 4)
        n0 = 32 * s
        aTx = self.ap.tile([128, NH, 32], _dt8("av"), tag="aTxs", name="aTx_s")
        for h in range(NH):
            hp, hc = 64 * (h % 2), h // 2
            pss = self.ps_sT()
            nc.tensor.matmul(
                pss[:31, :32],
                k_s[hp:hp + 64, hc, 128 * mt + mb:128 * mt + mb + 31],
                q_s[hp:hp + 64, hc, n0:n0 + 32],
                start=True, stop=True)
            nc.scalar.activation(out=aTx[mb:mb + 31, h, :],
                                 in_=pss[:31, :32], func=AF.Exp)
        return aTx

    def attn_spe_out(self, v_s, oT_s, s, aTx):
        nc = self.nc
        mt, mb = s // 4, 32 * (s % 4)
        n0 = 32 * s
        o_sb = self.ap.tile([128, 2, DIM], BF16, tag="osb", name="o_sb")
        for og in range(2):
            o_ps = self.ps_o()
            for hh in range(4):
                h = 4 * og + hh
                nc.tensor.matmul(
                    o_ps[:32, 65 * hh:65 * hh + 65],
                    aTx[mb:mb + 31, h, :],
                    v_s[mb:mb + 31, mt, h, :],
                    start=True, stop=True, tile_position=(mb, 0))
            rinv = self.sp.tile([128, 8], F32, tag="rinv", name="rinv")
            nc.vector.reciprocal(out=rinv[:32, 4 * og:4 * og + 4],
                                 in_=o_ps[:32, 64:260:65])
            src = o_ps[:32, :260].rearrange("p (h e) -> p h e", e=65)[:, :, 0:64]
            dst = o_sb[:32, 0, 256 * og:256 * og + 256].rearrange(
                "p (h e) -> p h e", e=64)
            nc.gpsimd.tensor_tensor(
                out=dst, in0=src,
                in1=rinv[:32, 4 * og:4 * og + 4, None].to_broadcast(
                    (32, 4, 64)),
                op=ALU.mult)
        pt4 = self.ps_tr4().rearrange("p (a b) -> p a b", a=4)[:, :, 0:32]
        for e in range(4):
            nc.tensor.transpose(pt4[:, e, :], o_sb[:32, 0, 128 * e:128 * (e + 1)],
                                self.ident[:32, :32])
        nc.any.tensor_copy(out=oT_s[:, :, n0:n0 + 32], in_=pt4)

    def resid_add(self, dt, psz):
        """X[:, dt, :] += psz, skipping the pad rows (98:128) of odd spatial
        tiles so pad tokens stay exactly zero (their k/v/q then vanish and
        DoubleRow AV over full 128-row chunks needs no value masking)."""
        nc, X = self.nc, self.X
        if dt < NT_SPA and dt % 2 == 1:
            nc.vector.tensor_tensor(out=X[0:98, dt, :], in0=X[0:98, dt, :],
                                    in1=psz[0:98, :], op=ALU.add)
        else:
            nc.vector.tensor_tensor(out=X[:, dt, :], in0=X[:, dt, :],
                                    in1=psz, op=ALU.add)

    def proj_group(self, blk, oT_g, base_tile):
        """Blanket residual: X[:, tile, :] += proj(oT). cls rows get the
        wrong (own-stream) delta here; fixed afterwards via cls_fix()."""
        nc, X = self.nc, self.X
        wproj = self.wblk[f"wproj{blk}"]
        for cc in range(4):
            psz = self.ps_mm()
            kcs, kw, pm = _kcs("proj")
            for e in kcs:
                nc.tensor.matmul(psz, oT_g[:, e:e + kw, 128 * cc:128 * (cc + 1)],
                                 wproj[:, e:e + kw, :], start=(e == 0),
                                 stop=(e == kcs[-1]), perf_mode=pm)
            if self.cfg["use_proj_b"]:
                nc.vector.tensor_tensor(
                    out=psz, in0=psz,
                    in1=self.bias["proj_b"][blk:blk + 1, :].to_broadcast((1, DIM)),
                    op=ALU.add)
            dt = base_tile + cc
            self.resid_add(dt, psz)

    def gather_cls(self, tx, ty):
        """DMA-gather the 32 cls rows of X into [16,512] tiles (spa, spe)."""
        nc, X = self.nc, self.X
        nc.sync.dma_start(out=tx, in_=X[0:1, 0:NT_SPA:2, :])
        for k in range(4):
            nc.sync.dma_start(out=ty[k:16:4, :],
                              in_=X[32 * k:32 * k + 1, NT_SPA:NT, :])
        # The framework's alias-wait for a tile written by N queued DMAs
        # covers only the first N-1 (observed off-by-one); this duplicate of
        # the last stripe makes the missed DMA a harmless re-write.
        nc.sync.dma_start(out=ty[3:16:4, :],
                          in_=X[96:97, NT_SPA:NT, :])

    def scatter_cls(self, tx, ty):
        nc, X = self.nc, self.X
        nc.sync.dma_start(out=X[0:1, 0:NT_SPA:2, :], in_=tx)
        for k in range(4):
            nc.sync.dma_start(out=X[32 * k:32 * k + 1, NT_SPA:NT, :],
                              in_=ty[k:16:4, :])

    def block(self, blk):
        nc, X, tc = self.nc, self.X, self.tc
        cfg = self.cfg
        # per-block weights
        self.wblk = {}
        for nm in ("wqk", "wv", "wproj", "wfc1", "wfc2"):
            key = nm + str(blk)
            shape = [128, 4, 2 * DIM] if nm == "wqk" else [128, 4, DIM]
            t = self.wp.tile(shape, _dt8({"wqk": "qkv", "wv": "qkv", "wproj": "proj", "wfc1": "fc1", "wfc2": "fc2"}[nm]), tag=nm, name="w_" + key)
            nc.sync.dma_start(out=t, in_=self.P[key][:])
            self.wblk[key] = t

        aff1 = self.bias["n1_wb"][blk] if cfg["use_n1"] else None
        stats1 = self.bp.tile([128, NT, 2], F32, tag="stats1", name="stats1")
        with nc.named_scope("ln_stats1"):
            self.ln_stats(stats1, 0, NT)

        # Phase A: LN+transpose ALL chunks up front (kept in SBUF) and swap
        # the cls columns between streams.  Reference cls semantics:
        #   xs2_cls = 2*xs_cls + y_attn_cls   (and symmetrically for ys)
        # Implemented without any cls gather/scatter DMAs: the attention
        # OUTPUT cls columns are swapped between oT_s and oT_g, the X cls
        # rows are doubled in place just before each blanket residual add,
        # and the blanket add then lands the other stream's delta.
        xlnT_spe = self.bp.tile([128, 4, DIM], _dt8("qkv"), tag="xlnTspe",
                                name="xlnT_spe")
        sc_ln = nc.named_scope("ln_apply1")
        sc_ln.__enter__()
        for lc in range(4):
            self.ln_apply_T(stats1, NT_SPA + lc, xlnT_spe, 128 * lc, aff1)
        xlnTs = [self.bp.tile([128, 4, DIM], _dt8("qkv"), tag=f"xlnT{g}",
                              name=f"xlnT_{g}") for g in range(8)]
        for g in range(8):
            for lc in range(4):
                self.ln_apply_T(stats1, 4 * g + lc, xlnTs[g], 128 * lc, aff1)
            for j in range(2):
                s = 2 * g + j
                c0 = 256 * j
                ctmp = self.sp.tile([128, 4, 1], _dt8("qkv"), tag="ctmp", name="ctmp")
                nc.any.tensor_copy(out=ctmp,
                                   in_=xlnT_spe[:, :, 32 * s:32 * s + 1])
                nc.any.tensor_copy(out=xlnT_spe[:, :, 32 * s:32 * s + 1],
                                   in_=xlnTs[g][:, :, c0:c0 + 1])
                nc.any.tensor_copy(out=xlnTs[g][:, :, c0:c0 + 1], in_=ctmp)

        sc_ln.__exit__(None, None, None)
        # Phase B: spe attention first, so its cls outputs are available for
        # the per-group output swaps below.
        q_s = self.gp.tile([128, 4, DIM], BF16, tag="qg", name="q_s")
        k_s = self.gp.tile([128, 4, DIM], BF16, tag="kg", name="k_s")
        v_s = self.gp.tile([128, 4, NH, 65], _dt8("av"), tag="vg", name="v_s")
        with nc.named_scope("spe_qkv"):
            self.qkv_group(blk, xlnT_spe, q_s, k_s, v_s, spa=False)
        oT_s = self.bp.tile([128, 4, DIM], _dt8("proj"), tag="oTs", name="oT_s")
        with nc.named_scope("spe_attn"):
            aTx_cur = self.attn_spe_scores(q_s, k_s, 0)
            for s in range(SB):
                nxt = (self.attn_spe_scores(q_s, k_s, s + 1)
                       if s + 1 < SB else None)
                self.attn_spe_out(v_s, oT_s, s, aTx_cur)
                aTx_cur = nxt

        # Phase C: spa groups, software-pipelined: group g's proj is
        # deferred until after group g+1's attention so the cls-swap DMAs
        # and the o-transposes are off the PE's critical path.
        prev_oT = None
        for g in range(8):
            xlnT_g = xlnTs[g]
            q_g = self.gp.tile([128, 4, DIM], BF16, tag="qg", name="q_g")
            k_g = self.gp.tile([128, 4, DIM], BF16, tag="kg", name="k_g")
            v_g = self.gp.tile([128, 4, NH, 65], _dt8("av"), tag="vg", name="v_g")
            with nc.named_scope("spa_qkv"):
                self.qkv_group(blk, xlnT_g, q_g, k_g, v_g)
            oT_g = self.gp.tile([128, 4, DIM], _dt8("proj"), tag="oTg", name="oT_g")
            with nc.named_scope("spa_attn"):
                osb = [self.attn_spa(q_g, k_g, v_g, j) for j in range(2)]
            if prev_oT is not None:
                pg, p_oT = prev_oT
                nc.vector.tensor_scalar(out=X[0:1, 4 * pg:4 * pg + 4:2, :],
                                        in0=X[0:1, 4 * pg:4 * pg + 4:2, :],
                                        scalar1=2.0, scalar2=None, op0=ALU.mult)
                with nc.named_scope("spa_proj"):
                    self.proj_group(blk, p_oT, 4 * pg)
            with nc.named_scope("spa_attn"):
                for j in range(2):
                    self.attn_spa_evac(osb[j], oT_g, j)
            # swap attention-output cls columns with the spe stream
            # (sample 2g+j lives at oT_s col 64g+32j)
            for j in range(2):
                sc = 64 * g + 32 * j
                otmp = self.sp.tile([128, 4, 1], _dt8("proj"), tag="otmp", name="otmp")
                nc.any.tensor_copy(out=otmp, in_=oT_s[:, :, sc:sc + 1])
                nc.any.tensor_copy(out=oT_s[:, :, sc:sc + 1],
                                   in_=oT_g[:, :, 256 * j:256 * j + 1])
                nc.any.tensor_copy(out=oT_g[:, :, 256 * j:256 * j + 1],
                                   in_=otmp)
            prev_oT = (g, oT_g)
        pg, p_oT = prev_oT
        nc.vector.tensor_scalar(out=X[0:1, 4 * pg:4 * pg + 4:2, :],
                                in0=X[0:1, 4 * pg:4 * pg + 4:2, :],
                                scalar1=2.0, scalar2=None, op0=ALU.mult)
        with nc.named_scope("spa_proj"):
            self.proj_group(blk, p_oT, 4 * pg)

        # Phase D: double spe cls rows, then the spe blanket add
        for k in range(4):
            nc.vector.tensor_scalar(out=X[32 * k:32 * k + 1, NT_SPA:NT, :],
                                    in0=X[32 * k:32 * k + 1, NT_SPA:NT, :],
                                    scalar1=2.0, scalar2=None, op0=ALU.mult)
        self.proj_group(blk, oT_s, NT_SPA)

        # ---- MLP ----
        if "mlp" in _SKIP:
            return
        wfc1, wfc2 = self.wblk[f"wfc1{blk}"], self.wblk[f"wfc2{blk}"]
        aff2 = self.bias["n2_wb"][blk] if cfg["use_n2"] else None
        stats2 = self.bp.tile([128, NT, 2], F32, tag="stats2", name="stats2")
        with nc.named_scope("ln_stats2"):
            self.ln_stats(stats2, 0, NT)
        fc1_b = self.bias.get("fc1_b")
        sc_mlp = nc.named_scope("mlp")
        sc_mlp.__enter__()

        def mlp_ln(r):
            x2T = self.gp.tile([128, 4, DIM], _dt8("fc1"), tag="x2T",
                               name="x2T")
            for lc in range(4):
                self.ln_apply_T(stats2, 4 * r + lc, x2T, 128 * lc, aff2)
            return x2T

        def mlp_fc1(x2T):
            h1T = self.gp.tile([128, 4, DIM], _dt8("fc2"), tag="h1T",
                               name="h1T")
            for m in range(4):
                psm = self.ps_mm()
                kcs, kw, pm = _kcs("fc1")
                for kc in kcs:
                    nc.tensor.matmul(psm,
                                     wfc1[:, kc:kc + kw, 128 * m:128 * (m + 1)],
                                     x2T[:, kc:kc + kw, :], start=(kc == 0),
                                     stop=(kc == kcs[-1]), perf_mode=pm)
                b1 = fc1_b[blk][:, m:m + 1] if fc1_b is not None else 0.0
                nc.scalar.activation(out=h1T[:, m, :], in_=psm, func=AF.Gelu,
                                     bias=b1, scale=1.0 / SCL)
            return h1T

        def mlp_fc2(r, h1T):
            for cc in range(4):
                psm = self.ps_mm()
                kcs, kw, pm = _kcs("fc2")
                for kc in kcs:
                    nc.tensor.matmul(psm,
                                     h1T[:, kc:kc + kw, 128 * cc:128 * (cc + 1)],
                                     wfc2[:, kc:kc + kw, :], start=(kc == 0),
                                     stop=(kc == kcs[-1]), perf_mode=pm)
                if cfg["use_fc2_b"]:
                    nc.vector.tensor_tensor(
                        out=psm, in0=psm,
                        in1=self.bias["fc2_b"][blk:blk + 1, :].to_broadcast(
                            (1, DIM)), op=ALU.add)
                self.resid_add(4 * r + cc, psm)

        # software pipeline: r+1's LN transposes fill the PE while r's last
        # gelu (ACT) finishes, so fc2 never head-of-line blocks the PE
        x2T_cur = mlp_ln(0)
        pend_fc2 = None
        for r in range(9):
            h1T = mlp_fc1(x2T_cur)
            if r < 8:
                x2T_cur = mlp_ln(r + 1)
            if pend_fc2 is not None:
                mlp_fc2(*pend_fc2)
            pend_fc2 = (r, h1T)
        mlp_fc2(*pend_fc2)
        sc_mlp.__exit__(None, None, None)

    # ------------------------------------------------------------ head
    def head(self):
        nc, X, cfg = self.nc, self.X, self.cfg
        xcf = [self.bp.tile([128, DIM], BF16, tag=f"xcf{h}", name=f"xcf{h}")[0:16]
               for h in range(2)]
        self.gather_cls(xcf[0], xcf[1])
        clsn = [self.bp.tile([128, DIM], BF16, tag=f"clsn{h}", name=f"clsn{h}")[0:16]
                for h in range(2)]
        for half in range(2):
            st = self.sp.tile([16, 6], F32, tag="lnstf", name="stf")
            mv = self.sp.tile([16, 2], F32, tag="fmv", name="mvf")
            nc.vector.bn_stats(out=st, in_=xcf[half])
            nc.vector.bn_aggr(out=mv, in_=st)
            nc.scalar.activation(out=mv[:, 1:2], in_=mv[:, 1:2], func=AF.Ln,
                                 bias=self.eps[0:16], scale=1.0)
            nc.scalar.activation(out=mv[:, 1:2], in_=mv[:, 1:2], func=AF.Exp,
                                 scale=-0.5)
            nc.vector.tensor_scalar(out=clsn[half], in0=xcf[half],
                                    scalar1=mv[:, 0:1], scalar2=mv[:, 1:2],
                                    op0=ALU.subtract, op1=ALU.mult)
            if cfg["use_nf"]:
                nc.vector.tensor_tensor(
                    out=clsn[half], in0=clsn[half],
                    in1=self.bias["nf_w"][half:half + 1, :].to_broadcast((1, DIM)),
                    op=ALU.mult)
                nc.vector.tensor_tensor(
                    out=clsn[half], in0=clsn[half],
                    in1=self.bias["nf_b"][half:half + 1, :].to_broadcast((1, DIM)),
                    op=ALU.add)
        clsT = self.bp.tile([128, 8, NCLS], BF16, tag="clsT", name="clsT")
        for half in range(2):
            for e in range(4):
                pt = self.ps_tr4()[:, 0:128]
                nc.tensor.transpose(pt[:, 0:16],
                                    clsn[half][:, 128 * e:128 * (e + 1)],
                                    self.ident[:16, :16])
                nc.any.tensor_copy(out=clsT[:, 4 * half + e, :], in_=pt[:, 0:16])
        psh = self.ps.tile([16, NCLS], F32, tag="mm", bufs=2, name="psh")
        for dc in range(8):
            nc.tensor.matmul(psh, clsT[:, dc, :], self.whead[:, dc, :],
                             start=(dc == 0), stop=(dc == 7))
        out_sb = self.bp.tile([128, NCLS], F32, tag="out_sb", name="out_sb")[0:16]
        if cfg["use_head_b"]:
            nc.vector.tensor_tensor(
                out=out_sb, in0=psh,
                in1=self.bias["head_b"][0:1, :].to_broadcast((1, NCLS)),
                op=ALU.add)
        else:
            nc.vector.tensor_copy(out=out_sb, in_=psh)
        nc.sync.dma_start(out=self.out_p[:], in_=out_sb)


# ---------------------------------------------------------------- entry
def kernel(**inputs):
    w = prepare_weights(inputs)
    nc = build_program(w)
    in_maps = make_in_maps(inputs, w)
    res = run_bass_kernel_spmd(nc, in_maps, list(range(NCORES)))
    out = np.concatenate([np.asarray(r["out"], np.float32)
                          for r in res.results], axis=0)
    return out


def run_traced(inputs):
    """For test.py: returns (out, BassKernelResults with exec_time_ns)."""
    w = prepare_weights(inputs)
    nc = build_program(w)
    in_maps = make_in_maps(inputs, w)
    res = run_bass_kernel_spmd(nc, in_maps, list(range(NCORES)), trace=True)
    out = np.concatenate([np.asarray(r["out"], np.float32)
                          for r in res.results], axis=0)
    return out, res

